# revision 3
# baseline (speedup 1.0000x reference)
"""Trainium2 Bass kernel: causal multi-head attention block (B=2, S=2048, D=4096,
32 heads x 128 head_dim, fp32, interleaved RoPE) tensor-parallel over heads on
8 NeuronCores, with an AllToAll to switch from head-parallel attention to
sequence-parallel output projection.

Per core i (4 heads = 512 features):
  phase Q/K : xq^T = wq_i @ x^T   (feature-major [512, 4096]), fused RoPE via a
              pair-swap permutation matmul + DVE combine with cos/sin tables.
  phase V   : v = x @ wv_i^T      (token-major [4096, 512]).
  attention : per (batch, head): scores^T = K^T_tile.T-matmul, masked exp on
              ScalarE, denominator via ones-matmul, PV matmul, normalize by
              1/denom broadcast (K=1 ones matmul).
  AllToAll  : per batch, head-slices -> token-slices across the 8 cores.
  phase WO  : out[tok_slice] = attn[tok_slice] @ wo^T, K accumulated in PSUM.

All PE-facing tensors are float32r (FP22 truncation in the PE at full speed).
"""

import sys

if "/opt/trn_rl_repo" not in sys.path:
    sys.path.insert(0, "/opt/trn_rl_repo")

import numpy as np

import concourse.bass as bass
import concourse.tile as tile
from concourse import bacc, mybir
from concourse.bass_utils import run_bass_kernel_spmd

F32 = mybir.dt.float32
F32R = mybir.dt.float32r

B, S, D = 2, 2048, 4096
H, HD = 32, 128
NCORES = 8
HPC = H // NCORES        # heads per core
F = HPC * HD             # 512 features per core
TOK = B * S              # 4096 tokens
KT = D // 128            # 32 contraction tiles
SCALE = 1.0 / float(np.sqrt(HD))
NEG = -1e30

_CACHE = {}


def _build():
    nc = bacc.Bacc("TRN2", target_bir_lowering=False, debug=False,
                   num_devices=NCORES)

    xT_d = nc.dram_tensor("xT", [D, TOK], F32R, kind="ExternalInput")
    wqT_d = nc.dram_tensor("wqT", [D, F], F32R, kind="ExternalInput")
    wkT_d = nc.dram_tensor("wkT", [D, F], F32R, kind="ExternalInput")
    wvT_d = nc.dram_tensor("wvT", [D, F], F32R, kind="ExternalInput")
    woT_d = nc.dram_tensor("woT", [D, D], F32R, kind="ExternalInput")
    cos_d = nc.dram_tensor("cosE", [128, S], F32, kind="ExternalInput")
    sin_d = nc.dram_tensor("sinE", [128, S], F32, kind="ExternalInput")
    mask_d = nc.dram_tensor("masks", [128, 4 * 512], F32, kind="ExternalInput")
    perm_d = nc.dram_tensor("permT", [128, 128], F32R, kind="ExternalInput")
    ones_d = nc.dram_tensor("ones", [128, 128], F32R, kind="ExternalInput")
    out_d = nc.dram_tensor("out", [TOK // NCORES, D], F32, kind="ExternalOutput")

    with tile.TileContext(nc) as tc:
        # DRAM scratch (tile-tracked so the scheduler orders phases/collectives)
        dram = tc.alloc_tile_pool(name="dram", bufs=1, space="DRAM")
        q_sp = dram.tile([F, TOK], F32R, name="q_sp")
        k_sp = dram.tile([F, TOK], F32R, name="k_sp")
        v_sp = dram.tile([B * HPC * S, HD], F32R, name="v_sp")
        a2a_in = [dram.tile([NCORES, F, 256], F32R, name=f"a2a_in{b}")
                  for b in range(B)]
        a2a_out = [dram.tile([NCORES, F, 256], F32R, name=f"a2a_out{b}")
                   for b in range(B)]

        with tc.tile_pool(name="consts", bufs=1) as cpool:
            cos_sb = cpool.tile([128, S], F32)
            nc.sync.dma_start(out=cos_sb[:], in_=cos_d[:, :])
            sin_sb = cpool.tile([128, S], F32)
            nc.sync.dma_start(out=sin_sb[:], in_=sin_d[:, :])
            mask_sb = cpool.tile([128, 4 * 512], F32)
            nc.sync.dma_start(out=mask_sb[:], in_=mask_d[:, :])
            perm_sb = cpool.tile([128, 128], F32R)
            nc.sync.dma_start(out=perm_sb[:], in_=perm_d[:, :])
            ones_sb = cpool.tile([128, 128], F32R)
            nc.sync.dma_start(out=ones_sb[:], in_=ones_d[:, :])

            # ---------------- phase Q / K: feature-major projections + RoPE
            for w_d, o_sp in ((wqT_d, q_sp), (wkT_d, k_sp)):
                with tc.tile_pool(name="wqk", bufs=1) as wpool, \
                     tc.tile_pool(name="xqk", bufs=2) as xpool, \
                     tc.tile_pool(name="psqk", bufs=2, space="PSUM") as pspool, \
                     tc.tile_pool(name="rotps", bufs=2, space="PSUM") as rotps, \
                     tc.tile_pool(name="ropew", bufs=3) as work:
                    w_sb = wpool.tile([128, KT * F], F32R, name="w_sb")
                    for kt in range(KT):
                        nc.sync.dma_start(
                            out=w_sb[:, kt * F:(kt + 1) * F],
                            in_=w_d[kt * 128:(kt + 1) * 128, :])
                    for nb in range(TOK // 256):
                        x_sb = xpool.tile([128, KT * 256], F32R, name="x_sb")
                        for kt in range(KT):
                            nc.sync.dma_start(
                                out=x_sb[:, kt * 256:(kt + 1) * 256],
                                in_=xT_d[kt * 128:(kt + 1) * 128,
                                         nb * 256:(nb + 1) * 256])
                        s0 = (nb * 256) % S
                        for m in range(HPC):
                            ps = pspool.tile([128, 256], F32, name="ps")
                            for kt in range(KT):
                                nc.tensor.matmul(
                                    ps[:],
                                    w_sb[:, kt * F + m * 128: kt * F + (m + 1) * 128],
                                    x_sb[:, kt * 256:(kt + 1) * 256],
                                    start=(kt == 0), stop=(kt == KT - 1))
                            raw = work.tile([128, 256], F32R, tag="raw", name="raw")
                            nc.scalar.copy(raw[:], ps[:])
                            rot = rotps.tile([128, 256], F32, name="rot")
                            nc.tensor.matmul(rot[:], perm_sb[:], raw[:],
                                             start=True, stop=True)
                            t1 = work.tile([128, 256], F32, tag="t1", name="t1")
                            nc.vector.tensor_mul(
                                t1[:], raw[:], cos_sb[:, s0:s0 + 256])
                            t2 = work.tile([128, 256], F32, tag="t2", name="t2")
                            nc.vector.tensor_mul(
                                t2[:], rot[:], sin_sb[:, s0:s0 + 256])
                            qf = work.tile([128, 256], F32R, tag="qf", name="qf")
                            nc.vector.tensor_add(qf[:], t1[:], t2[:])
                            nc.sync.dma_start(
                                out=o_sp[m * 128:(m + 1) * 128,
                                         nb * 256:(nb + 1) * 256],
                                in_=qf[:])

            # ---------------- phase V: token-major projection
            with tc.tile_pool(name="wv", bufs=1) as wpool, \
                 tc.tile_pool(name="xv", bufs=2) as xpool, \
                 tc.tile_pool(name="psv", bufs=2, space="PSUM") as pspool, \
                 tc.tile_pool(name="vout", bufs=3) as vout:
                wv_sb = wpool.tile([128, KT * F], F32R, name="wv_sb")
                for kt in range(KT):
                    nc.sync.dma_start(
                        out=wv_sb[:, kt * F:(kt + 1) * F],
                        in_=wvT_d[kt * 128:(kt + 1) * 128, :])
                for nb in range(TOK // 256):
                    x_sb = xpool.tile([128, KT * 256], F32R, name="x_sb")
                    for kt in range(KT):
                        nc.sync.dma_start(
                            out=x_sb[:, kt * 256:(kt + 1) * 256],
                            in_=xT_d[kt * 128:(kt + 1) * 128,
                                     nb * 256:(nb + 1) * 256])
                    for mt in range(2):
                        ps = pspool.tile([128, F], F32, name="ps")
                        for kt in range(KT):
                            nc.tensor.matmul(
                                ps[:],
                                x_sb[:, kt * 256 + mt * 128: kt * 256 + (mt + 1) * 128],
                                wv_sb[:, kt * F:(kt + 1) * F],
                                start=(kt == 0), stop=(kt == KT - 1))
                        v_sb = vout.tile([128, F], F32R, name="v_sb")
                        nc.scalar.copy(v_sb[:], ps[:])
                        tok0 = nb * 256 + mt * 128
                        b, sx = tok0 // S, tok0 % S
                        for h in range(HPC):
                            nc.sync.dma_start(
                                out=v_sp[(b * HPC + h) * S + sx:
                                         (b * HPC + h) * S + sx + 128, :],
                                in_=v_sb[:, h * 128:(h + 1) * 128])

            # ---------------- attention per (batch, head) + per-batch AllToAll
            with tc.tile_pool(name="aqkv", bufs=2) as apool, \
                 tc.tile_pool(name="exw", bufs=6) as expool, \
                 tc.tile_pool(name="amisc", bufs=2) as misc, \
                 tc.tile_pool(name="scps", bufs=2, space="PSUM") as scps, \
                 tc.tile_pool(name="pvps", bufs=2, space="PSUM") as pvps, \
                 tc.tile_pool(name="dps", bufs=2, space="PSUM") as dps, \
                 tc.tile_pool(name="bcps", bufs=1, space="PSUM") as bcps:
                for b in range(B):
                    for h in range(HPC):
                        q_sb = apool.tile([128, S], F32R, tag="q", name="q_sb")
                        nc.sync.dma_start(
                            out=q_sb[:],
                            in_=q_sp[h * 128:(h + 1) * 128, b * S:(b + 1) * S])
                        k_sb = apool.tile([128, S], F32R, tag="k", name="k_sb")
                        nc.sync.dma_start(
                            out=k_sb[:],
                            in_=k_sp[h * 128:(h + 1) * 128, b * S:(b + 1) * S])
                        v_sb = apool.tile([128, (S // 128) * 128], F32R,
                                          tag="v", name="v_sb")
                        for st in range(S // 128):
                            base = (b * HPC + h) * S + st * 128
                            nc.sync.dma_start(
                                out=v_sb[:, st * 128:(st + 1) * 128],
                                in_=v_sp[base:base + 128, :])
                        for qt in range(4):
                            nkt = 4 * qt + 4
                            dsum = dps.tile([1, 512], F32, name="dsum")
                            pv = pvps.tile([128, 512], F32, name="pv")
                            for kt in range(nkt):
                                sc = scps.tile([128, 512], F32, name="sc")
                                nc.tensor.matmul(
                                    sc[:], k_sb[:, kt * 128:(kt + 1) * 128],
                                    q_sb[:, qt * 512:(qt + 1) * 512],
                                    start=True, stop=True)
                                r = kt - 4 * qt
                                ex = expool.tile([128, 512], F32R, tag="ex",
                                                 name="ex")
                                if r >= 0:
                                    scm = expool.tile([128, 512], F32,
                                                      tag="scm", name="scm")
                                    nc.vector.tensor_add(
                                        scm[:], sc[:],
                                        mask_sb[:, r * 512:(r + 1) * 512])
                                    src = scm
                                else:
                                    src = sc
                                nc.scalar.activation(
                                    ex[:], src[:],
                                    mybir.ActivationFunctionType.Exp,
                                    scale=SCALE)
                                nc.tensor.matmul(
                                    dsum[:], ones_sb[:, 0:1], ex[:],
                                    start=(kt == 0), stop=(kt == nkt - 1))
                                nc.tensor.matmul(
                                    pv[:], v_sb[:, kt * 128:(kt + 1) * 128],
                                    ex[:],
                                    start=(kt == 0), stop=(kt == nkt - 1))
                            rec = misc.tile([1, 512], F32R, tag="rec", name="rec")
                            with nc.allow_low_precision(
                                    reason="1/denom consumed by f32r matmul"):
                                nc.vector.reciprocal(rec[:], dsum[:])
                            bc = bcps.tile([128, 512], F32, name="bc")
                            nc.tensor.matmul(bc[:], ones_sb[0:1, :], rec[:],
                                             start=True, stop=True)
                            bc_sb = misc.tile([128, 512], F32, tag="bcsb",
                                              name="bc_sb")
                            nc.vector.tensor_copy(bc_sb[:], bc[:])
                            at = misc.tile([128, 512], F32R, tag="at", name="at")
                            nc.vector.tensor_mul(at[:], pv[:], bc_sb[:])
                            for u in range(2):
                                nc.sync.dma_start(
                                    out=a2a_in[b][2 * qt + u,
                                                  h * 128:(h + 1) * 128, :],
                                    in_=at[:, u * 256:(u + 1) * 256])
                    nc.gpsimd.collective_compute(
                        "AllToAll", mybir.AluOpType.bypass,
                        replica_groups=[list(range(NCORES))],
                        ins=[a2a_in[b][:]], outs=[a2a_out[b][:]])

            # ---------------- phase WO: sequence-parallel output projection
            with tc.tile_pool(name="atp", bufs=1) as atpool, \
                 tc.tile_pool(name="wop", bufs=2) as wopool, \
                 tc.tile_pool(name="pswo", bufs=2, space="PSUM") as pspool, \
                 tc.tile_pool(name="wout", bufs=3) as wout:
                at_sb = []
                for b in range(B):
                    t = atpool.tile([128, KT * 256], F32R, tag=f"at{b}",
                                    name=f"at_sb{b}")
                    for kt in range(KT):
                        jj, off = (kt * 128) // F, (kt * 128) % F
                        nc.sync.dma_start(
                            out=t[:, kt * 256:(kt + 1) * 256],
                            in_=a2a_out[b][jj, off:off + 128, :])
                    at_sb.append(t)
                for n in range(D // 256):
                    wo_blk = wopool.tile([128, KT * 256], F32R, name="wo_blk")
                    for kt in range(KT):
                        nc.sync.dma_start(
                            out=wo_blk[:, kt * 256:(kt + 1) * 256],
                            in_=woT_d[kt * 128:(kt + 1) * 128,
                                      n * 256:(n + 1) * 256])
                    for b in range(B):
                        for mt in range(2):
                            ps = pspool.tile([128, 256], F32, name="ps")
                            for kt in range(KT):
                                nc.tensor.matmul(
                                    ps[:],
                                    at_sb[b][:, kt * 256 + mt * 128:
                                             kt * 256 + (mt + 1) * 128],
                                    wo_blk[:, kt * 256:(kt + 1) * 256],
                                    start=(kt == 0), stop=(kt == KT - 1))
                            o_sb = wout.tile([128, 256], F32, name="o_sb")
                            nc.scalar.copy(o_sb[:], ps[:])
                            nc.sync.dma_start(
                                out=out_d[b * 256 + mt * 128:
                                          b * 256 + (mt + 1) * 128,
                                          n * 256:(n + 1) * 256],
                                in_=o_sb[:])

    nc.compile()
    return nc


def _host_inputs(x, wq, wk, wv, wo):
    x = np.asarray(x, dtype=np.float32)
    xT = np.ascontiguousarray(x.reshape(TOK, D).T)
    woT = np.ascontiguousarray(np.asarray(wo, dtype=np.float32).T)

    inv = (1.0 / (10000.0 ** (np.arange(0, HD, 2, dtype=np.float64) / HD)))
    fr = np.outer(np.arange(S, dtype=np.float64), inv)       # [S, HD/2]
    cosE = np.repeat(np.cos(fr).T, 2, axis=0).astype(np.float32)
    sinE = np.repeat(np.sin(fr).T, 2, axis=0).astype(np.float32)

    masks = np.zeros([128, 4 * 512], dtype=np.float32)
    qi = np.arange(512)
    pi = np.arange(128)
    for r in range(4):
        masks[:, r * 512:(r + 1) * 512][qi[None, :] < (r * 128 + pi)[:, None]] = NEG

    permT = np.zeros([128, 128], dtype=np.float32)
    ii = np.arange(0, 128, 2)
    permT[ii + 1, ii] = -1.0
    permT[ii, ii + 1] = 1.0

    ones = np.ones([128, 128], dtype=np.float32)

    maps = []
    for i in range(NCORES):
        sl = slice(i * F, (i + 1) * F)
        maps.append(dict(
            xT=xT,
            wqT=np.ascontiguousarray(np.asarray(wq, np.float32)[sl, :].T),
            wkT=np.ascontiguousarray(np.asarray(wk, np.float32)[sl, :].T),
            wvT=np.ascontiguousarray(np.asarray(wv, np.float32)[sl, :].T),
            woT=woT,
            cosE=cosE, sinE=sinE, masks=masks, permT=permT, ones=ones,
        ))
    return maps


def kernel(x, start_pos, wq, wk, wv, wo, _trace=False):
    if "nc" not in _CACHE:
        _CACHE["nc"] = _build()
    nc = _CACHE["nc"]
    maps = _host_inputs(x, wq, wk, wv, wo)
    res = run_bass_kernel_spmd(nc, maps, core_ids=list(range(NCORES)),
                               trace=_trace)
    _CACHE["last"] = res
    full = np.empty([TOK, D], dtype=np.float32)
    for j in range(NCORES):
        o = res.results[j]["out"]
        full[j * 256:(j + 1) * 256] = o[:256]
        full[S + j * 256: S + (j + 1) * 256] = o[256:]
    return full.reshape(B, S, D)
